# revision 11
# baseline (speedup 1.0000x reference)
"""Bass/Trainium2 kernel for nn_Attention_369367188096 (sparse_attention).

Reference computation (B=2, N=4096, IN_DIM=1024, DIM=1024, HEADS=8, d=128):
    qkv = x @ W_qkv ; split into q,k,v per head
    dots = (q @ k^T) * DIM**-0.5 ; masked on top-left [2048,2048] block
    attn = softmax(dots) ; out = attn @ v ; out @ W_out + b_out

Sharding across 8 NeuronCores: core i handles batch b=i//4 and heads
(2*(i%4), 2*(i%4)+1).  Each core computes a partial output
x[b]-rows x DIM using its two heads' slice of W_out (row-sharded);
the host sums 4 partials per batch and adds b_out.

v3: PE-bound design, every non-S matmul stream shrunk and all engines
kept busy end-to-end:
- On real TRN2 a matmul costs out-free-size cycles regardless of dtype;
  fp8 DoubleRow's win is contracting TWO 128-deep k-planes per stream.
  PV and the softmax denominator contract j (4096) -> DR pairs halve
  them; Q/K projections contract IN_DIM (1024) -> DR over c-chunk pairs
  (x and W_q/W_k shipped as fp8; V projection stays bf16 for accuracy).
- S = K^T Q contracts only d=128, so it stays bf16 (no DR win exists).
- exp on ScalarE writes fp8 directly; mask is an fp8 0/1 multiply on
  VectorE; 1/den via DVE reciprocal_approx_fast.
- Single instruction stream interleaves the phases: V-projection chunks,
  head-1 Q/K projection units and output-projection halves are spliced
  into the attention pair loop's PE slack (in-order engine queues make
  emission order = execution order), so ScalarE's exp pipe starts ~25us
  in and the PE never idles long enough to drop out of its top p-state.
"""

import os
import sys

for _p in ("/opt/trn_rl_repo", "/root/.axon_site/_ro/trn_rl_repo"):
    if os.path.isdir(_p) and _p not in sys.path:
        sys.path.insert(0, _p)

from collections import deque
from contextlib import ExitStack

import ml_dtypes
import numpy as np

import concourse.bass as bass
import concourse.bacc as bacc
import concourse.mybir as mybir
import concourse.tile as tile
from concourse.bass_utils import run_bass_kernel_spmd

BF16 = mybir.dt.bfloat16
FP8 = mybir.dt.float8e4
F32 = mybir.dt.float32
DR = mybir.MatmulPerfMode.DoubleRow
P = 128          # partitions
IN_DIM = 1024    # model in dim
OUT_DIM = 1024   # model out dim
DH = 128         # head dim
NH = 2           # heads per core
FD = 512         # matmul moving free dim
N_FULL = 4096    # sequence length
MM_FULL = 2048   # masked block size
SCALE = 1024 ** -0.5
N_CORES = 8


def build_nc(n=N_FULL, mm=MM_FULL):
    """Build the per-core Bass program (SPMD: same program, per-core data)."""
    CI = IN_DIM // P          # 8 input-dim chunks
    CP = CI // 2              # c-chunk pairs for DR projections (4)
    JC = n // P               # key chunks (32)
    IG = n // FD              # query groups of 512 (8)
    MJ = mm // P              # masked key chunks (16)
    MG = mm // FD             # masked query groups (4)
    assert MJ % 2 == 0 and JC % 2 == 0
    AF = mybir.ActivationFunctionType

    nc = bacc.Bacc("TRN2", target_bir_lowering=False, debug=False)
    wq_d = nc.dram_tensor("wq", [P, CI * NH * DH], FP8, kind="ExternalInput")
    wk_d = nc.dram_tensor("wk", [P, CI * NH * DH], FP8, kind="ExternalInput")
    wv_d = nc.dram_tensor("wv", [P, CI * NH * DH], BF16, kind="ExternalInput")
    wo_d = nc.dram_tensor("wo", [P, NH * OUT_DIM], BF16, kind="ExternalInput")
    x8_d = nc.dram_tensor("x8", [IN_DIM, n], FP8, kind="ExternalInput")
    xt_d = nc.dram_tensor("xt", [IN_DIM, n], BF16, kind="ExternalInput")
    mk_d = nc.dram_tensor("maskt", [mm, mm], FP8, kind="ExternalInput")
    out_d = nc.dram_tensor("part", [n, OUT_DIM], BF16, kind="ExternalOutput")

    NQ = n // 4               # x8 DMA quarter width
    x8_v = x8_d.rearrange("(c p) n -> c p n", p=P)
    xt_v = xt_d.rearrange("(c p) n -> c p n", p=P)
    mk_v = mk_d.rearrange("(j p) i -> p j i", p=P)
    out_v = out_d.rearrange("(t p) o -> t p o", p=P)

    with tile.TileContext(nc) as tc, ExitStack() as ctx:
        const = ctx.enter_context(tc.tile_pool(name="const", bufs=1))

        # Resident inputs, DMA'd in dependency order: fp8 QK path first
        # (it gates the first matmul), then the bf16 V path.
        wq8 = const.tile([P, CI, NH * DH], FP8, tag="wq")
        wk8 = const.tile([P, CI, NH * DH], FP8, tag="wk")
        for t, d_ in ((wq8, wq_d), (wk8, wk_d)):
            nc.sync.dma_start(t[:], d_.rearrange("p (a b) -> p a b", a=CI))
        # x8 lands in n-quarters, all c-chunks of a quarter first, so the
        # first projection units can start ~6us in instead of waiting for
        # whole 0.5MB chunk transfers on single DMA engines.
        x8 = const.tile([P, CI, n], FP8, tag="x8")
        for q in range(4):
            for c in range(CI):
                nc.sync.dma_start(
                    x8[:, c, q * NQ:(q + 1) * NQ],
                    x8_v[c][:, q * NQ:(q + 1) * NQ])
        wv = const.tile([P, CI, NH * DH], BF16, tag="wv")
        wo = const.tile([P, NH, OUT_DIM], BF16, tag="wo")
        for t, d_ in ((wv, wv_d), (wo, wo_d)):
            nc.sync.dma_start(t[:], d_.rearrange("p (a b) -> p a b", a=t.shape[1]))
        xt = [const.tile([P, n], BF16, tag=f"xt{c}", name=f"xt{c}") for c in range(CI)]
        for c in range(CI):
            nc.sync.dma_start(xt[c][:], xt_v[c])
        ones8 = const.tile([P, 2, P], FP8, tag="ones")
        nc.vector.memset(ones8[:], 1.0)

        # Resident intermediates
        qt = [const.tile([P, n], BF16, tag=f"qt{h}", name=f"qt{h}") for h in range(NH)]
        kt = [const.tile([P, n], BF16, tag=f"kt{h}", name=f"kt{h}") for h in range(NH)]
        vb8 = const.tile([P, JC, NH * DH], FP8, tag="vb")      # [j, jc, (h d)]
        ot = [const.tile([P, n], BF16, tag=f"ot{h}", name=f"ot{h}") for h in range(NH)]

        pst = ctx.enter_context(tc.tile_pool(name="pst", bufs=2, space="PSUM"))
        px = ctx.enter_context(tc.tile_pool(name="px", bufs=2, space="PSUM"))
        po = ctx.enter_context(tc.tile_pool(name="po", bufs=1, space="PSUM"))
        pd = ctx.enter_context(tc.tile_pool(name="pd", bufs=1, space="PSUM"))
        att = ctx.enter_context(tc.tile_pool(name="att", bufs=5))
        mkp = ctx.enter_context(tc.tile_pool(name="mkp", bufs=4))
        obp = ctx.enter_context(tc.tile_pool(name="obp", bufs=3))

        # ---- emission units (each: a few PE streams + a DVE eviction) ----
        def emit_qk_g(h, w8, dst, g):
            # one i-group of a Q^T/K^T projection: DR over c-chunk pairs
            ps = px.tile([P, FD], F32, tag="u", name="psu")
            for cp in range(CP):
                nc.tensor.matmul(
                    ps[:], w8[:, 2 * cp:2 * cp + 2, h * DH:(h + 1) * DH],
                    x8[:, 2 * cp:2 * cp + 2, g * FD:(g + 1) * FD],
                    start=(cp == 0), stop=(cp == CP - 1), perf_mode=DR,
                )
            nc.vector.tensor_copy(dst[:, g * FD:(g + 1) * FD], ps[:])

        def emit_v_chunk(t):
            # one 128-row chunk of V for both heads (bf16), evicted to fp8
            ps = px.tile([P, FD], F32, tag="u", name="psu")
            pv = ps[:, :NH * DH]
            for c in range(CI):
                nc.tensor.matmul(
                    pv, xt[c][:, t * P:(t + 1) * P], wv[:, c, :],
                    start=(c == 0), stop=(c == CI - 1),
                )
            nc.vector.tensor_copy(vb8[:, t, :], pv)

        def emit_outproj_half(t, nf):
            ps = px.tile([P, FD], F32, tag="u", name="psu")
            for h in range(NH):
                nc.tensor.matmul(
                    ps[:], ot[h][:, t * P:(t + 1) * P],
                    wo[:, h, nf * FD:(nf + 1) * FD],
                    start=(h == 0), stop=(h == NH - 1),
                )
            ob = obp.tile([P, FD], BF16, tag="ob", name="ob")
            nc.vector.tensor_copy(ob[:], ps[:])
            nc.sync.dma_start(out_v[t][:, nf * FD:(nf + 1) * FD], ob[:])

        # splice queues, drained on a fixed schedule inside the pair loop
        qk1_units = deque()
        op_units = deque()

        # ---- head: Q/K projections for head 0, first V chunks ----
        for w8, dst in ((wq8, qt[0]), (wk8, kt[0])):
            for g in range(IG):
                emit_qk_g(0, w8, dst, g)
        V_UPFRONT = 4
        for t in range(V_UPFRONT):
            emit_v_chunk(t)
        v_todo = deque(range(V_UPFRONT, JC))
        for w8, dst in ((wq8, qt[1]), (wk8, kt[1])):
            for g in range(IG):
                qk1_units.append(lambda h=1, w8=w8, dst=dst, g=g:
                                 emit_qk_g(h, w8, dst, g))

        # ---- attention pair loop (phases interleaved via splice pops) ----
        NP2 = JC // 2
        h0_pair = [0]

        def pop_splices(h, g, jp):
            if h == 0 and g == 0:
                # V chunks just-in-time, one pair ahead of this PV stream
                for _ in range(2):
                    if v_todo:
                        emit_v_chunk(v_todo.popleft())
            elif h == 0:
                # head-1 Q/K projection spread evenly over these 112 pairs
                h0_pair[0] += 1
                if h0_pair[0] % 7 == 3 and qk1_units:
                    qk1_units.popleft()()
            else:
                # output projection halves, one group's lag behind finalize
                if jp % 2 == 1 and op_units:
                    op_units.popleft()()

        for h in range(NH):
            for g in range(IG):
                gs = g * FD
                oacc = po.tile([P, FD], F32, tag="po")   # [d, i] accum
                dacc = pd.tile([P, FD], F32, tag="pd")   # bcast denom accum

                def emit_s(jp):
                    st = pst.tile([P, 2, FD], F32, tag="st", name="st")
                    for u in range(2):
                        nc.tensor.matmul(
                            st[:, u, :],
                            kt[h][:, (2 * jp + u) * P:(2 * jp + u + 1) * P],
                            qt[h][:, gs:gs + FD],
                            start=True, stop=True,
                        )
                    return st

                st_next = emit_s(0)
                for jp in range(NP2):
                    st = st_next
                    if jp + 1 < NP2:
                        st_next = emit_s(jp + 1)
                    pop_splices(h, g, jp)
                    j0 = 2 * jp
                    masked = j0 + 1 < MJ and g < MG
                    pt2 = att.tile([P, 2, FD], FP8, tag="pt")
                    nc.scalar.activation(pt2[:], st[:], AF.Exp, scale=SCALE)
                    if masked:
                        mt2 = mkp.tile([P, 2, FD], FP8, tag="mt")
                        nc.sync.dma_start(
                            mt2[:], mk_v[:, j0:j0 + 2, gs:gs + FD])
                        nc.vector.tensor_mul(
                            out=pt2[:], in0=pt2[:], in1=mt2[:])
                    last_pair = jp == NP2 - 1
                    nc.tensor.matmul(
                        oacc[:], vb8[:, j0:j0 + 2, h * DH:(h + 1) * DH],
                        pt2[:], start=(jp == 0), stop=last_pair,
                        perf_mode=DR,
                    )
                    nc.tensor.matmul(
                        dacc[:], ones8[:], pt2[:],
                        start=(jp == 0), stop=last_pair,
                        perf_mode=DR,
                    )
                # free the single-bank accumulators ASAP, then normalize
                osb = att.tile([P, FD], F32, tag="osb", name="osb", bufs=2)
                dsb = att.tile([P, FD], F32, tag="dsb", name="dsb", bufs=2)
                nc.vector.tensor_copy(osb[:], oacc[:])
                nc.vector.tensor_copy(dsb[:], dacc[:])
                rec = att.tile([P, FD], F32, tag="rec", name="rec", bufs=2)
                nc.vector.reciprocal_approx_fast(rec[:], dsb[:])
                nc.vector.tensor_mul(
                    out=ot[h][:, gs:gs + FD], in0=osb[:], in1=rec[:])
                if h == NH - 1:
                    for t in range(4 * g, 4 * g + 4):
                        for nf in range(OUT_DIM // FD):
                            op_units.append(lambda t=t, nf=nf:
                                            emit_outproj_half(t, nf))

        # ---- tail: drain remaining spliced work ----
        while v_todo:
            emit_v_chunk(v_todo.popleft())
        while qk1_units:
            qk1_units.popleft()()
        while op_units:
            op_units.popleft()()

    nc.compile()
    return nc


def make_core_inputs(x, W_qkv, W_out, mask, n=N_FULL, mm=MM_FULL):
    """Host-side shard prep: per-core input dicts (pre-transposed).

    W slices are delivered in the on-chip layout ([128, c*h*d] with the
    IN_DIM chunk index between partition and column) so the DMA is dense.
    x ships twice: fp8 (Q/K DoubleRow path) and bf16 (V path).
    """
    bf = ml_dtypes.bfloat16
    f8 = ml_dtypes.float8_e4m3
    B = x.shape[0]
    CI = IN_DIM // P
    xt_b = [np.ascontiguousarray(x[b].T) for b in range(B)]
    xt_bf = [t.astype(bf) for t in xt_b]
    xt_f8 = [t.astype(f8) for t in xt_b]
    maskt = np.ascontiguousarray(mask[0, 0, :mm, :mm].T).astype(f8)

    def wlayout(w, dt):  # [IN_DIM, NH*DH] -> [P, CI*NH*DH]
        return np.ascontiguousarray(
            w.reshape(CI, P, NH * DH).transpose(1, 0, 2).reshape(P, -1)
        ).astype(dt)

    cores_per_b = N_CORES // B
    in_maps = []
    for core in range(N_CORES):
        b = core // cores_per_b
        h0 = NH * (core % cores_per_b)
        qs, ks, vs = (W_qkv[:, o + h0 * DH: o + (h0 + NH) * DH]
                      for o in (0, OUT_DIM, 2 * OUT_DIM))
        wo_slice = W_out[h0 * DH:(h0 + NH) * DH, :]  # [NH*DH, OUT_DIM]
        wo_l = np.ascontiguousarray(
            wo_slice.reshape(NH, P, OUT_DIM).transpose(1, 0, 2).reshape(P, -1)
        ).astype(bf)
        in_maps.append({
            "xt": xt_bf[b],
            "x8": xt_f8[b],
            "wq": wlayout(qs, f8),
            "wk": wlayout(ks, f8),
            "wv": wlayout(vs, bf),
            "wo": wo_l,
            "maskt": maskt,
        })
    return in_maps


_NC_CACHE = {}


def _get_nc(n=N_FULL, mm=MM_FULL):
    key = (n, mm)
    if key not in _NC_CACHE:
        _NC_CACHE[key] = build_nc(n, mm)
    return _NC_CACHE[key]


def run(x, W_qkv, W_out, b_out, mask, trace=False, **trace_kwargs):
    nc = _get_nc()
    in_maps = make_core_inputs(x, W_qkv, W_out, mask)
    res = run_bass_kernel_spmd(
        nc, in_maps, list(range(N_CORES)), trace=trace, **trace_kwargs
    )
    B = x.shape[0]
    cores_per_b = N_CORES // B
    out = np.zeros((B, N_FULL, OUT_DIM), np.float32)
    for core in range(N_CORES):
        out[core // cores_per_b] += np.asarray(
            res.results[core]["part"], dtype=np.float32)
    out += np.asarray(b_out, np.float32)
    return out, res


def kernel(x, W_qkv, W_out, b_out, mask, max_mask=MM_FULL, **_ignored):
    x = np.asarray(x, np.float32)
    W_qkv = np.asarray(W_qkv, np.float32)
    W_out = np.asarray(W_out, np.float32)
    b_out = np.asarray(b_out, np.float32)
    mask = np.asarray(mask)
    out, _ = run(x, W_qkv, W_out, b_out, mask)
    return out


# revision 18
# speedup vs baseline: 1.0125x; 1.0125x over previous
"""Bass/Trainium2 kernel for nn_Attention_369367188096 (sparse_attention).

Reference computation (B=2, N=4096, IN_DIM=1024, DIM=1024, HEADS=8, d=128):
    qkv = x @ W_qkv ; split into q,k,v per head
    dots = (q @ k^T) * DIM**-0.5 ; masked on top-left [2048,2048] block
    attn = softmax(dots) ; out = attn @ v ; out @ W_out + b_out

Sharding across 8 NeuronCores: core i handles batch b=i//4 and heads
(2*(i%4), 2*(i%4)+1).  Each core computes a partial output
x[b]-rows x DIM using its two heads' slice of W_out (row-sharded);
the host sums 4 partials per batch and adds b_out.

v3: PE-bound design, every non-S matmul stream shrunk and all engines
kept busy end-to-end:
- On real TRN2 a matmul costs out-free-size cycles regardless of dtype;
  fp8 DoubleRow's win is contracting TWO 128-deep k-planes per stream.
  PV and the softmax denominator contract j (4096) -> DR pairs halve
  them; Q/K projections contract IN_DIM (1024) -> DR over c-chunk pairs
  (x and W_q/W_k shipped as fp8; V projection stays bf16 for accuracy).
- S = K^T Q contracts only d=128, so it stays bf16 (no DR win exists).
- exp on ScalarE writes fp8 directly; mask is an fp8 0/1 multiply on
  VectorE; 1/den via DVE reciprocal_approx_fast.
- Single instruction stream interleaves the phases: V-projection chunks,
  head-1 Q/K projection units and output-projection halves are spliced
  into the attention pair loop's PE slack (in-order engine queues make
  emission order = execution order), so ScalarE's exp pipe starts ~25us
  in and the PE never idles long enough to drop out of its top p-state.
"""

import os
import sys

for _p in ("/opt/trn_rl_repo", "/root/.axon_site/_ro/trn_rl_repo"):
    if os.path.isdir(_p) and _p not in sys.path:
        sys.path.insert(0, _p)

from collections import deque
from contextlib import ExitStack

import ml_dtypes
import numpy as np

import concourse.bass as bass
import concourse.bacc as bacc
import concourse.mybir as mybir
import concourse.tile as tile
from concourse.bass_utils import run_bass_kernel_spmd

BF16 = mybir.dt.bfloat16
FP8 = mybir.dt.float8e4
F32 = mybir.dt.float32
DR = mybir.MatmulPerfMode.DoubleRow
P = 128          # partitions
IN_DIM = 1024    # model in dim
OUT_DIM = 1024   # model out dim
DH = 128         # head dim
NH = 2           # heads per core
FD = 512         # matmul moving free dim
N_FULL = 4096    # sequence length
MM_FULL = 2048   # masked block size
SCALE = 1024 ** -0.5
N_CORES = 8


def build_nc(n=N_FULL, mm=MM_FULL):
    """Build the per-core Bass program (SPMD: same program, per-core data)."""
    CI = IN_DIM // P          # 8 input-dim chunks
    CP = CI // 2              # c-chunk pairs for DR projections (4)
    JC = n // P               # key chunks (32)
    IG = n // FD              # query groups of 512 (8)
    MJ = mm // P              # masked key chunks (16)
    MG = mm // FD             # masked query groups (4)
    assert MJ % 2 == 0 and JC % 2 == 0
    AF = mybir.ActivationFunctionType

    nc = bacc.Bacc("TRN2", target_bir_lowering=False, debug=False)
    wq_d = nc.dram_tensor("wq", [P, CI * NH * DH], FP8, kind="ExternalInput")
    wk_d = nc.dram_tensor("wk", [P, CI * NH * DH], FP8, kind="ExternalInput")
    wv_d = nc.dram_tensor("wv", [P, CI * NH * DH], BF16, kind="ExternalInput")
    wo_d = nc.dram_tensor("wo", [P, NH * OUT_DIM], BF16, kind="ExternalInput")
    x8_d = nc.dram_tensor("x8", [IN_DIM, n], FP8, kind="ExternalInput")
    mk_d = nc.dram_tensor("maskt", [mm, mm], FP8, kind="ExternalInput")
    out_d = nc.dram_tensor("part", [n, OUT_DIM], BF16, kind="ExternalOutput")

    NQ = n // 4               # x8 DMA quarter width
    x8_v = x8_d.rearrange("(c p) n -> c p n", p=P)
    mk_v = mk_d.rearrange("(j p) i -> p j i", p=P)
    out_v = out_d.rearrange("(t p) o -> t p o", p=P)

    with tile.TileContext(nc) as tc, ExitStack() as ctx:
        const = ctx.enter_context(tc.tile_pool(name="const", bufs=1))

        # Resident inputs. Transfers are sliced small so they parallelize
        # across the 16 DMA engines (a single dma_start runs on ONE engine
        # at ~22GB/s), and the issue stream is split across the two HWDGE
        # queues (Sync + Scalar) because each dma_start costs ~0.6us of
        # issue time on its queue.  Sync: W slices + x8 first half.
        # Scalar (idle until the first exp): x8 second half.
        wq8 = const.tile([P, CI, NH * DH], FP8, tag="wq")
        wk8 = const.tile([P, CI, NH * DH], FP8, tag="wk")
        wq_v = wq_d.rearrange("p (a b) -> p a b", a=CI)
        wk_v = wk_d.rearrange("p (a b) -> p a b", a=CI)
        x8 = const.tile([P, CI, n], FP8, tag="x8")
        for c in range(CI):
            nc.sync.dma_start(wq8[:, c, :], wq_v[:, c, :])
        for q in (0,):
            for c in range(CI):
                nc.sync.dma_start(
                    x8[:, c, q * NQ:(q + 1) * NQ],
                    x8_v[c][:, q * NQ:(q + 1) * NQ])
        for c in range(0, CI, 4):
            nc.sync.dma_start(wk8[:, c:c + 4, :], wk_v[:, c:c + 4, :])
        for q in (1,):
            for c in range(CI):
                nc.sync.dma_start(
                    x8[:, c, q * NQ:(q + 1) * NQ],
                    x8_v[c][:, q * NQ:(q + 1) * NQ])
        for q in (2, 3):
            for c in range(CI):
                nc.scalar.dma_start(
                    x8[:, c, q * NQ:(q + 1) * NQ],
                    x8_v[c][:, q * NQ:(q + 1) * NQ])
        wv = const.tile([P, CI, NH * DH], BF16, tag="wv")
        wo = const.tile([P, NH, OUT_DIM], BF16, tag="wo")
        for c in range(0, CI, 4):
            nc.sync.dma_start(
                wv[:, c:c + 4, :],
                wv_d.rearrange("p (a b) -> p a b", a=CI)[:, c:c + 4, :])
        nc.sync.dma_start(wo[:], wo_d.rearrange("p (a b) -> p a b", a=NH))
        ones8 = const.tile([P, 2, P], FP8, tag="ones")
        nc.vector.memset(ones8[:], 1.0)

        # Resident intermediates
        qt = [const.tile([P, n], BF16, tag=f"qt{h}", name=f"qt{h}") for h in range(NH)]
        kt = [const.tile([P, n], BF16, tag=f"kt{h}", name=f"kt{h}") for h in range(NH)]
        vb8 = const.tile([P, JC, NH * DH], FP8, tag="vb")      # [j, jc, (h d)]
        ot = [const.tile([P, n], BF16, tag=f"ot{h}", name=f"ot{h}") for h in range(NH)]

        pst = ctx.enter_context(tc.tile_pool(name="pst", bufs=2, space="PSUM"))
        px = ctx.enter_context(tc.tile_pool(name="px", bufs=2, space="PSUM"))
        po = ctx.enter_context(tc.tile_pool(name="po", bufs=1, space="PSUM"))
        pd = ctx.enter_context(tc.tile_pool(name="pd", bufs=1, space="PSUM"))
        att = ctx.enter_context(tc.tile_pool(name="att", bufs=5))
        mkp = ctx.enter_context(tc.tile_pool(name="mkp", bufs=4))
        obp = ctx.enter_context(tc.tile_pool(name="obp", bufs=3))

        # ---- emission units (each: a few PE streams + a DVE eviction) ----
        def emit_qk_g(h, w8, dst, g):
            # one i-group of a Q^T/K^T projection: DR over c-chunk pairs
            ps = px.tile([P, FD], F32, tag="u", name="psu")
            for cp in range(CP):
                nc.tensor.matmul(
                    ps[:], w8[:, 2 * cp:2 * cp + 2, h * DH:(h + 1) * DH],
                    x8[:, 2 * cp:2 * cp + 2, g * FD:(g + 1) * FD],
                    start=(cp == 0), stop=(cp == CP - 1), perf_mode=DR,
                )
            nc.vector.tensor_copy(dst[:, g * FD:(g + 1) * FD], ps[:])

        def emit_v_chunk(t):
            # one 128-row chunk of V for both heads, evicted to fp8.
            # lhsT is the fp8 x (the PE takes mixed fp8 weights x bf16
            # ifmap); wv stays bf16 so V only carries x's quantization.
            ps = px.tile([P, FD], F32, tag="u", name="psu")
            pv = ps[:, :NH * DH]
            for c in range(CI):
                nc.tensor.matmul(
                    pv, x8[:, c, t * P:(t + 1) * P], wv[:, c, :],
                    start=(c == 0), stop=(c == CI - 1),
                )
            nc.vector.tensor_copy(vb8[:, t, :], pv)

        def emit_outproj_half(t, nf):
            ps = px.tile([P, FD], F32, tag="u", name="psu")
            for h in range(NH):
                nc.tensor.matmul(
                    ps[:], ot[h][:, t * P:(t + 1) * P],
                    wo[:, h, nf * FD:(nf + 1) * FD],
                    start=(h == 0), stop=(h == NH - 1),
                )
            ob = obp.tile([P, FD], BF16, tag="ob", name="ob")
            nc.vector.tensor_copy(ob[:], ps[:])
            nc.gpsimd.dma_start(out_v[t][:, nf * FD:(nf + 1) * FD], ob[:])

        # splice queues, drained on a fixed schedule inside the pair loop
        qk1_units = deque()
        op_units = deque()

        # ---- head: Q/K projections for head 0, first V chunks ----
        for w8, dst in ((wq8, qt[0]), (wk8, kt[0])):
            for g in range(IG):
                emit_qk_g(0, w8, dst, g)
        V_UPFRONT = 4
        for t in range(V_UPFRONT):
            emit_v_chunk(t)
        v_todo = deque(range(V_UPFRONT, JC))
        for w8, dst in ((wq8, qt[1]), (wk8, kt[1])):
            for g in range(IG):
                qk1_units.append(lambda h=1, w8=w8, dst=dst, g=g:
                                 emit_qk_g(h, w8, dst, g))

        # ---- attention pair loop (phases interleaved via splice pops) ----
        NP2 = JC // 2
        h0_pair = [0]

        def pop_splices(h, g, jp):
            if h == 0 and g == 0:
                # V chunks just-in-time, one pair ahead of this PV stream
                for _ in range(2):
                    if v_todo:
                        emit_v_chunk(v_todo.popleft())
            elif h == 0:
                # head-1 Q/K projection spread evenly over these 112 pairs
                h0_pair[0] += 1
                if h0_pair[0] % 7 == 3 and qk1_units:
                    qk1_units.popleft()()
            else:
                # output projection halves, one group's lag behind finalize
                if jp % 2 == 1 and op_units:
                    op_units.popleft()()

        for h in range(NH):
            for g in range(IG):
                gs = g * FD
                oacc = po.tile([P, FD], F32, tag="po")   # [d, i] accum
                dacc = pd.tile([P, FD], F32, tag="pd")   # bcast denom accum

                def emit_s(jp):
                    st = pst.tile([P, 2, FD], F32, tag="st", name="st")
                    for u in range(2):
                        nc.tensor.matmul(
                            st[:, u, :],
                            kt[h][:, (2 * jp + u) * P:(2 * jp + u + 1) * P],
                            qt[h][:, gs:gs + FD],
                            start=True, stop=True,
                        )
                    return st

                st_next = emit_s(0)
                for jp in range(NP2):
                    st = st_next
                    if jp + 1 < NP2:
                        st_next = emit_s(jp + 1)
                    pop_splices(h, g, jp)
                    j0 = 2 * jp
                    masked = j0 + 1 < MJ and g < MG
                    pt2 = att.tile([P, 2, FD], FP8, tag="pt")
                    nc.scalar.activation(pt2[:], st[:], AF.Exp, scale=SCALE)
                    if masked:
                        mt2 = mkp.tile([P, 2, FD], FP8, tag="mt")
                        nc.gpsimd.dma_start(
                            mt2[:], mk_v[:, j0:j0 + 2, gs:gs + FD])
                        nc.vector.tensor_mul(
                            out=pt2[:], in0=pt2[:], in1=mt2[:])
                    last_pair = jp == NP2 - 1
                    nc.tensor.matmul(
                        oacc[:], vb8[:, j0:j0 + 2, h * DH:(h + 1) * DH],
                        pt2[:], start=(jp == 0), stop=last_pair,
                        perf_mode=DR,
                    )
                    nc.tensor.matmul(
                        dacc[:], ones8[:], pt2[:],
                        start=(jp == 0), stop=last_pair,
                        perf_mode=DR,
                    )
                # free the single-bank accumulators ASAP, then normalize
                osb = att.tile([P, FD], F32, tag="osb", name="osb", bufs=2)
                dsb = att.tile([P, FD], F32, tag="dsb", name="dsb", bufs=2)
                nc.vector.tensor_copy(osb[:], oacc[:])
                nc.vector.tensor_copy(dsb[:], dacc[:])
                rec = att.tile([P, FD], F32, tag="rec", name="rec", bufs=2)
                nc.vector.reciprocal_approx_fast(rec[:], dsb[:])
                nc.vector.tensor_mul(
                    out=ot[h][:, gs:gs + FD], in0=osb[:], in1=rec[:])
                if h == NH - 1:
                    for t in range(4 * g, 4 * g + 4):
                        for nf in range(OUT_DIM // FD):
                            op_units.append(lambda t=t, nf=nf:
                                            emit_outproj_half(t, nf))

        # ---- tail: drain remaining spliced work ----
        while v_todo:
            emit_v_chunk(v_todo.popleft())
        while qk1_units:
            qk1_units.popleft()()
        while op_units:
            op_units.popleft()()

    nc.compile()
    return nc


def make_core_inputs(x, W_qkv, W_out, mask, n=N_FULL, mm=MM_FULL):
    """Host-side shard prep: per-core input dicts (pre-transposed).

    W slices are delivered in the on-chip layout ([128, c*h*d] with the
    IN_DIM chunk index between partition and column) so the DMA is dense.
    x ships twice: fp8 (Q/K DoubleRow path) and bf16 (V path).
    """
    bf = ml_dtypes.bfloat16
    f8 = ml_dtypes.float8_e4m3
    B = x.shape[0]
    CI = IN_DIM // P
    xt_f8 = [np.ascontiguousarray(x[b].T).astype(f8) for b in range(B)]
    maskt = np.ascontiguousarray(mask[0, 0, :mm, :mm].T).astype(f8)

    def wlayout(w, dt):  # [IN_DIM, NH*DH] -> [P, CI*NH*DH]
        return np.ascontiguousarray(
            w.reshape(CI, P, NH * DH).transpose(1, 0, 2).reshape(P, -1)
        ).astype(dt)

    cores_per_b = N_CORES // B
    in_maps = []
    for core in range(N_CORES):
        b = core // cores_per_b
        h0 = NH * (core % cores_per_b)
        qs, ks, vs = (W_qkv[:, o + h0 * DH: o + (h0 + NH) * DH]
                      for o in (0, OUT_DIM, 2 * OUT_DIM))
        wo_slice = W_out[h0 * DH:(h0 + NH) * DH, :]  # [NH*DH, OUT_DIM]
        wo_l = np.ascontiguousarray(
            wo_slice.reshape(NH, P, OUT_DIM).transpose(1, 0, 2).reshape(P, -1)
        ).astype(bf)
        in_maps.append({
            "x8": xt_f8[b],
            "wq": wlayout(qs, f8),
            "wk": wlayout(ks, f8),
            "wv": wlayout(vs, bf),
            "wo": wo_l,
            "maskt": maskt,
        })
    return in_maps


_NC_CACHE = {}


def _get_nc(n=N_FULL, mm=MM_FULL):
    key = (n, mm)
    if key not in _NC_CACHE:
        _NC_CACHE[key] = build_nc(n, mm)
    return _NC_CACHE[key]


def run(x, W_qkv, W_out, b_out, mask, trace=False, **trace_kwargs):
    nc = _get_nc()
    in_maps = make_core_inputs(x, W_qkv, W_out, mask)
    res = run_bass_kernel_spmd(
        nc, in_maps, list(range(N_CORES)), trace=trace, **trace_kwargs
    )
    B = x.shape[0]
    cores_per_b = N_CORES // B
    out = np.zeros((B, N_FULL, OUT_DIM), np.float32)
    for core in range(N_CORES):
        out[core // cores_per_b] += np.asarray(
            res.results[core]["part"], dtype=np.float32)
    out += np.asarray(b_out, np.float32)
    return out, res


def kernel(x, W_qkv, W_out, b_out, mask, max_mask=MM_FULL, **_ignored):
    x = np.asarray(x, np.float32)
    W_qkv = np.asarray(W_qkv, np.float32)
    W_out = np.asarray(W_out, np.float32)
    b_out = np.asarray(b_out, np.float32)
    mask = np.asarray(mask)
    out, _ = run(x, W_qkv, W_out, b_out, mask)
    return out


# revision 22
# speedup vs baseline: 1.0237x; 1.0110x over previous
"""Bass/Trainium2 kernel for nn_Attention_369367188096 (sparse_attention).

Reference computation (B=2, N=4096, IN_DIM=1024, DIM=1024, HEADS=8, d=128):
    qkv = x @ W_qkv ; split into q,k,v per head
    dots = (q @ k^T) * DIM**-0.5 ; masked on top-left [2048,2048] block
    attn = softmax(dots) ; out = attn @ v ; out @ W_out + b_out

Sharding across 8 NeuronCores: core i handles batch b=i//4 and heads
(2*(i%4), 2*(i%4)+1).  Each core computes a partial output
x[b]-rows x DIM using its two heads' slice of W_out (row-sharded);
the host sums 4 partials per batch and adds b_out.

v3: PE-bound design, every non-S matmul stream shrunk and all engines
kept busy end-to-end:
- On real TRN2 a matmul costs out-free-size cycles regardless of dtype;
  fp8 DoubleRow's win is contracting TWO 128-deep k-planes per stream.
  PV and the softmax denominator contract j (4096) -> DR pairs halve
  them; Q/K projections contract IN_DIM (1024) -> DR over c-chunk pairs
  (x and W_q/W_k shipped as fp8; V projection stays bf16 for accuracy).
- S = K^T Q contracts only d=128, so it stays bf16 (no DR win exists).
- exp on ScalarE writes fp8 directly; mask is an fp8 0/1 multiply on
  VectorE; 1/den via DVE reciprocal_approx_fast.
- Single instruction stream interleaves the phases: V-projection chunks,
  head-1 Q/K projection units and output-projection halves are spliced
  into the attention pair loop's PE slack (in-order engine queues make
  emission order = execution order), so ScalarE's exp pipe starts ~25us
  in and the PE never idles long enough to drop out of its top p-state.
"""

import os
import sys

for _p in ("/opt/trn_rl_repo", "/root/.axon_site/_ro/trn_rl_repo"):
    if os.path.isdir(_p) and _p not in sys.path:
        sys.path.insert(0, _p)

from collections import deque
from contextlib import ExitStack

import ml_dtypes
import numpy as np

import concourse.bass as bass
import concourse.bacc as bacc
import concourse.mybir as mybir
import concourse.tile as tile
from concourse.bass_utils import run_bass_kernel_spmd

BF16 = mybir.dt.bfloat16
FP8 = mybir.dt.float8e4
F32 = mybir.dt.float32
DR = mybir.MatmulPerfMode.DoubleRow
P = 128          # partitions
IN_DIM = 1024    # model in dim
OUT_DIM = 1024   # model out dim
DH = 128         # head dim
NH = 2           # heads per core
FD = 512         # matmul moving free dim
N_FULL = 4096    # sequence length
MM_FULL = 2048   # masked block size
SCALE = 1024 ** -0.5
N_CORES = 8


def build_nc(n=N_FULL, mm=MM_FULL):
    """Build the per-core Bass program (SPMD: same program, per-core data)."""
    CI = IN_DIM // P          # 8 input-dim chunks
    CP = CI // 2              # c-chunk pairs for DR projections (4)
    JC = n // P               # key chunks (32)
    IG = n // FD              # query groups of 512 (8)
    MJ = mm // P              # masked key chunks (16)
    MG = mm // FD             # masked query groups (4)
    assert MJ % 2 == 0 and JC % 2 == 0
    AF = mybir.ActivationFunctionType

    nc = bacc.Bacc("TRN2", target_bir_lowering=False, debug=False)
    wq_d = nc.dram_tensor("wq", [P, CI * NH * DH], FP8, kind="ExternalInput")
    wk_d = nc.dram_tensor("wk", [P, CI * NH * DH], FP8, kind="ExternalInput")
    wv_d = nc.dram_tensor("wv", [P, CI * NH * DH], BF16, kind="ExternalInput")
    wo_d = nc.dram_tensor("wo", [P, NH * OUT_DIM], BF16, kind="ExternalInput")
    x8_d = nc.dram_tensor("x8", [IN_DIM, n], FP8, kind="ExternalInput")
    mk_d = nc.dram_tensor("maskt", [mm, mm], FP8, kind="ExternalInput")
    out_d = nc.dram_tensor("part", [n, OUT_DIM], BF16, kind="ExternalOutput")

    NQ = n // 4               # x8 DMA quarter width
    x8_v = x8_d.rearrange("(c p) n -> c p n", p=P)
    mk_v = mk_d.rearrange("(j p) i -> p j i", p=P)
    out_v = out_d.rearrange("(t p) o -> t p o", p=P)

    with tile.TileContext(nc) as tc, ExitStack() as ctx:
        const = ctx.enter_context(tc.tile_pool(name="const", bufs=1))

        # Resident inputs. Transfers are sliced small so they parallelize
        # across the 16 DMA engines (a single dma_start runs on ONE engine
        # at ~22GB/s), and the issue stream is split across the two HWDGE
        # queues (Sync + Scalar) because each dma_start costs ~0.6us of
        # issue time on its queue.  Sync: W slices + x8 first half.
        # Scalar (idle until the first exp): x8 second half.
        # x8 lives as four separate n-quarter tiles: the Tile dependency
        # tracker is per-tile, so a consumer of quarter q must not be made
        # to wait on later quarters' transfers.
        wq8 = const.tile([P, CI, NH * DH], FP8, tag="wq")
        wk8 = const.tile([P, CI, NH * DH], FP8, tag="wk")
        wq_v = wq_d.rearrange("p (a b) -> p a b", a=CI)
        wk_v = wk_d.rearrange("p (a b) -> p a b", a=CI)
        x8q = [const.tile([P, CI, NQ], FP8, tag=f"x8q{q}", name=f"x8q{q}")
               for q in range(4)]
        for c in range(CI):
            nc.sync.dma_start(wq8[:, c, :], wq_v[:, c, :])
        for c in range(CI):
            nc.sync.dma_start(
                x8q[0][:, c, :], x8_v[c][:, 0:NQ])
        for c in range(0, CI, 4):
            nc.sync.dma_start(wk8[:, c:c + 4, :], wk_v[:, c:c + 4, :])
        for c in range(CI):
            nc.sync.dma_start(
                x8q[1][:, c, :], x8_v[c][:, NQ:2 * NQ])
        for q in (2, 3):
            for c in range(CI):
                nc.scalar.dma_start(
                    x8q[q][:, c, :], x8_v[c][:, q * NQ:(q + 1) * NQ])

        def x8_slice(cp, lo, width):
            # [P, 2, width] view of columns lo..lo+width at c-pair cp
            q = lo // NQ
            assert (lo + width - 1) // NQ == q
            return x8q[q][:, 2 * cp:2 * cp + 2, lo - q * NQ:lo - q * NQ + width]
        wv = const.tile([P, CI, NH * DH], BF16, tag="wv")
        wo = const.tile([P, NH, OUT_DIM], BF16, tag="wo")
        for c in range(0, CI, 4):
            nc.sync.dma_start(
                wv[:, c:c + 4, :],
                wv_d.rearrange("p (a b) -> p a b", a=CI)[:, c:c + 4, :])
        nc.sync.dma_start(wo[:], wo_d.rearrange("p (a b) -> p a b", a=NH))
        ones8 = const.tile([P, 2, P], FP8, tag="ones")
        nc.vector.memset(ones8[:], 1.0)

        # Resident intermediates
        qt = [const.tile([P, n], BF16, tag=f"qt{h}", name=f"qt{h}") for h in range(NH)]
        kt = [const.tile([P, n], BF16, tag=f"kt{h}", name=f"kt{h}") for h in range(NH)]
        vb8 = const.tile([P, JC, NH * DH], FP8, tag="vb")      # [j, jc, (h d)]
        ot = [const.tile([P, n], BF16, tag=f"ot{h}", name=f"ot{h}") for h in range(NH)]

        pst = ctx.enter_context(tc.tile_pool(name="pst", bufs=2, space="PSUM"))
        px = ctx.enter_context(tc.tile_pool(name="px", bufs=2, space="PSUM"))
        po = ctx.enter_context(tc.tile_pool(name="po", bufs=1, space="PSUM"))
        pd = ctx.enter_context(tc.tile_pool(name="pd", bufs=1, space="PSUM"))
        att = ctx.enter_context(tc.tile_pool(name="att", bufs=5))
        mkp = ctx.enter_context(tc.tile_pool(name="mkp", bufs=4))
        obp = ctx.enter_context(tc.tile_pool(name="obp", bufs=3))

        # ---- emission units (each: a few PE streams + a DVE eviction) ----
        def emit_qk_g(h, w8, dst, g):
            # one i-group of a Q^T/K^T projection: DR over c-chunk pairs
            ps = px.tile([P, FD], F32, tag="u", name="psu")
            for cp in range(CP):
                nc.tensor.matmul(
                    ps[:], w8[:, 2 * cp:2 * cp + 2, h * DH:(h + 1) * DH],
                    x8_slice(cp, g * FD, FD),
                    start=(cp == 0), stop=(cp == CP - 1), perf_mode=DR,
                )
            nc.vector.tensor_copy(dst[:, g * FD:(g + 1) * FD], ps[:])

        def emit_v_chunk(t):
            # one 128-row chunk of V for both heads, evicted to fp8.
            # lhsT is the fp8 x (the PE takes mixed fp8 weights x bf16
            # ifmap); wv stays bf16 so V only carries x's quantization.
            ps = px.tile([P, FD], F32, tag="u", name="psu")
            pv = ps[:, :NH * DH]
            q, col = t * P // NQ, t * P % NQ
            for c in range(CI):
                nc.tensor.matmul(
                    pv, x8q[q][:, c, col:col + P], wv[:, c, :],
                    start=(c == 0), stop=(c == CI - 1),
                )
            nc.vector.tensor_copy(vb8[:, t, :], pv)

        def emit_outproj_half(t, nf):
            ps = px.tile([P, FD], F32, tag="u", name="psu")
            for h in range(NH):
                nc.tensor.matmul(
                    ps[:], ot[h][:, t * P:(t + 1) * P],
                    wo[:, h, nf * FD:(nf + 1) * FD],
                    start=(h == 0), stop=(h == NH - 1),
                )
            ob = obp.tile([P, FD], BF16, tag="ob", name="ob")
            nc.vector.tensor_copy(ob[:], ps[:])
            nc.sync.dma_start(out_v[t][:, nf * FD:(nf + 1) * FD], ob[:])

        # splice queues, drained on a fixed schedule inside the pair loop
        qk1_units = deque()
        op_units = deque()

        # ---- head: Q/K projections for head 0, first V chunks ----
        for w8, dst in ((wq8, qt[0]), (wk8, kt[0])):
            for g in range(IG):
                emit_qk_g(0, w8, dst, g)
        V_UPFRONT = 4
        for t in range(V_UPFRONT):
            emit_v_chunk(t)
        v_todo = deque(range(V_UPFRONT, JC))
        for w8, dst in ((wq8, qt[1]), (wk8, kt[1])):
            for g in range(IG):
                qk1_units.append(lambda h=1, w8=w8, dst=dst, g=g:
                                 emit_qk_g(h, w8, dst, g))

        # ---- attention pair loop (phases interleaved via splice pops) ----
        NP2 = JC // 2
        h0_pair = [0]

        def pop_splices(h, g, jp):
            if h == 0 and g == 0:
                # V chunks just-in-time, one pair ahead of this PV stream
                for _ in range(2):
                    if v_todo:
                        emit_v_chunk(v_todo.popleft())
            elif h == 0:
                # head-1 Q/K projection spread evenly over these 112 pairs
                h0_pair[0] += 1
                if h0_pair[0] % 7 == 3 and qk1_units:
                    qk1_units.popleft()()
            else:
                # output projection halves, one group's lag behind finalize
                if jp % 2 == 1 and op_units:
                    op_units.popleft()()

        for h in range(NH):
            for g in range(IG):
                gs = g * FD
                oacc = po.tile([P, FD], F32, tag="po")   # [d, i] accum
                dacc = pd.tile([P, FD], F32, tag="pd")   # bcast denom accum

                def emit_s(jp):
                    st = pst.tile([P, 2, FD], F32, tag="st", name="st")
                    for u in range(2):
                        nc.tensor.matmul(
                            st[:, u, :],
                            kt[h][:, (2 * jp + u) * P:(2 * jp + u + 1) * P],
                            qt[h][:, gs:gs + FD],
                            start=True, stop=True,
                        )
                    return st

                st_next = emit_s(0)
                for jp in range(NP2):
                    st = st_next
                    if jp + 1 < NP2:
                        st_next = emit_s(jp + 1)
                    pop_splices(h, g, jp)
                    j0 = 2 * jp
                    masked = j0 + 1 < MJ and g < MG
                    pt2 = att.tile([P, 2, FD], FP8, tag="pt")
                    nc.scalar.activation(pt2[:], st[:], AF.Exp, scale=SCALE)
                    if masked:
                        mt2 = mkp.tile([P, 2, FD], FP8, tag="mt")
                        nc.gpsimd.dma_start(
                            mt2[:], mk_v[:, j0:j0 + 2, gs:gs + FD])
                        nc.vector.tensor_mul(
                            out=pt2[:], in0=pt2[:], in1=mt2[:])
                    last_pair = jp == NP2 - 1
                    nc.tensor.matmul(
                        oacc[:], vb8[:, j0:j0 + 2, h * DH:(h + 1) * DH],
                        pt2[:], start=(jp == 0), stop=last_pair,
                        perf_mode=DR,
                    )
                    nc.tensor.matmul(
                        dacc[:], ones8[:], pt2[:],
                        start=(jp == 0), stop=last_pair,
                        perf_mode=DR,
                    )
                # free the single-bank accumulators ASAP, then normalize
                osb = att.tile([P, FD], F32, tag="osb", name="osb", bufs=2)
                dsb = att.tile([P, FD], F32, tag="dsb", name="dsb", bufs=2)
                nc.vector.tensor_copy(osb[:], oacc[:])
                nc.vector.tensor_copy(dsb[:], dacc[:])
                rec = att.tile([P, FD], F32, tag="rec", name="rec", bufs=2)
                nc.vector.reciprocal_approx_fast(rec[:], dsb[:])
                nc.vector.tensor_mul(
                    out=ot[h][:, gs:gs + FD], in0=osb[:], in1=rec[:])
                if h == NH - 1:
                    for t in range(4 * g, 4 * g + 4):
                        for nf in range(OUT_DIM // FD):
                            op_units.append(lambda t=t, nf=nf:
                                            emit_outproj_half(t, nf))

        # ---- tail: drain remaining spliced work ----
        while v_todo:
            emit_v_chunk(v_todo.popleft())
        while qk1_units:
            qk1_units.popleft()()
        while op_units:
            op_units.popleft()()

    nc.compile()
    return nc


def make_core_inputs(x, W_qkv, W_out, mask, n=N_FULL, mm=MM_FULL):
    """Host-side shard prep: per-core input dicts (pre-transposed).

    W slices are delivered in the on-chip layout ([128, c*h*d] with the
    IN_DIM chunk index between partition and column) so the DMA is dense.
    x ships twice: fp8 (Q/K DoubleRow path) and bf16 (V path).
    """
    bf = ml_dtypes.bfloat16
    f8 = ml_dtypes.float8_e4m3
    B = x.shape[0]
    CI = IN_DIM // P
    xt_f8 = [np.ascontiguousarray(x[b].T).astype(f8) for b in range(B)]
    maskt = np.ascontiguousarray(mask[0, 0, :mm, :mm].T).astype(f8)

    def wlayout(w, dt):  # [IN_DIM, NH*DH] -> [P, CI*NH*DH]
        return np.ascontiguousarray(
            w.reshape(CI, P, NH * DH).transpose(1, 0, 2).reshape(P, -1)
        ).astype(dt)

    cores_per_b = N_CORES // B
    in_maps = []
    for core in range(N_CORES):
        b = core // cores_per_b
        h0 = NH * (core % cores_per_b)
        qs, ks, vs = (W_qkv[:, o + h0 * DH: o + (h0 + NH) * DH]
                      for o in (0, OUT_DIM, 2 * OUT_DIM))
        wo_slice = W_out[h0 * DH:(h0 + NH) * DH, :]  # [NH*DH, OUT_DIM]
        wo_l = np.ascontiguousarray(
            wo_slice.reshape(NH, P, OUT_DIM).transpose(1, 0, 2).reshape(P, -1)
        ).astype(bf)
        in_maps.append({
            "x8": xt_f8[b],
            "wq": wlayout(qs, f8),
            "wk": wlayout(ks, f8),
            "wv": wlayout(vs, bf),
            "wo": wo_l,
            "maskt": maskt,
        })
    return in_maps


_NC_CACHE = {}


def _get_nc(n=N_FULL, mm=MM_FULL):
    key = (n, mm)
    if key not in _NC_CACHE:
        _NC_CACHE[key] = build_nc(n, mm)
    return _NC_CACHE[key]


def run(x, W_qkv, W_out, b_out, mask, trace=False, **trace_kwargs):
    nc = _get_nc()
    in_maps = make_core_inputs(x, W_qkv, W_out, mask)
    res = run_bass_kernel_spmd(
        nc, in_maps, list(range(N_CORES)), trace=trace, **trace_kwargs
    )
    B = x.shape[0]
    cores_per_b = N_CORES // B
    out = np.zeros((B, N_FULL, OUT_DIM), np.float32)
    for core in range(N_CORES):
        out[core // cores_per_b] += np.asarray(
            res.results[core]["part"], dtype=np.float32)
    out += np.asarray(b_out, np.float32)
    return out, res


def kernel(x, W_qkv, W_out, b_out, mask, max_mask=MM_FULL, **_ignored):
    x = np.asarray(x, np.float32)
    W_qkv = np.asarray(W_qkv, np.float32)
    W_out = np.asarray(W_out, np.float32)
    b_out = np.asarray(b_out, np.float32)
    mask = np.asarray(mask)
    out, _ = run(x, W_qkv, W_out, b_out, mask)
    return out


# revision 26
# speedup vs baseline: 1.0316x; 1.0077x over previous
"""Bass/Trainium2 kernel for nn_Attention_369367188096 (sparse_attention).

Reference computation (B=2, N=4096, IN_DIM=1024, DIM=1024, HEADS=8, d=128):
    qkv = x @ W_qkv ; split into q,k,v per head
    dots = (q @ k^T) * DIM**-0.5 ; masked on top-left [2048,2048] block
    attn = softmax(dots) ; out = attn @ v ; out @ W_out + b_out

Sharding across 8 NeuronCores: core i handles batch b=i//4 and heads
(2*(i%4), 2*(i%4)+1).  Each core computes a partial output
x[b]-rows x DIM using its two heads' slice of W_out (row-sharded);
the host sums 4 partials per batch and adds b_out.

v3: PE-bound design, every non-S matmul stream shrunk and all engines
kept busy end-to-end:
- On real TRN2 a matmul costs out-free-size cycles regardless of dtype;
  fp8 DoubleRow's win is contracting TWO 128-deep k-planes per stream.
  PV and the softmax denominator contract j (4096) -> DR pairs halve
  them; Q/K projections contract IN_DIM (1024) -> DR over c-chunk pairs
  (x and W_q/W_k shipped as fp8; V projection stays bf16 for accuracy).
- S = K^T Q contracts only d=128, so it stays bf16 (no DR win exists).
- exp on ScalarE writes fp8 directly; mask is an fp8 0/1 multiply on
  VectorE; 1/den via DVE reciprocal_approx_fast.
- Single instruction stream interleaves the phases: V-projection chunks,
  head-1 Q/K projection units and output-projection halves are spliced
  into the attention pair loop's PE slack (in-order engine queues make
  emission order = execution order), so ScalarE's exp pipe starts ~25us
  in and the PE never idles long enough to drop out of its top p-state.
"""

import os
import sys

for _p in ("/opt/trn_rl_repo", "/root/.axon_site/_ro/trn_rl_repo"):
    if os.path.isdir(_p) and _p not in sys.path:
        sys.path.insert(0, _p)

from collections import deque
from contextlib import ExitStack

import ml_dtypes
import numpy as np

import concourse.bass as bass
import concourse.bacc as bacc
import concourse.mybir as mybir
import concourse.tile as tile
from concourse.bass_utils import run_bass_kernel_spmd

BF16 = mybir.dt.bfloat16
FP8 = mybir.dt.float8e4
F32 = mybir.dt.float32
DR = mybir.MatmulPerfMode.DoubleRow
P = 128          # partitions
IN_DIM = 1024    # model in dim
OUT_DIM = 1024   # model out dim
DH = 128         # head dim
NH = 2           # heads per core
FD = 512         # matmul moving free dim
N_FULL = 4096    # sequence length
MM_FULL = 2048   # masked block size
SCALE = 1024 ** -0.5
N_CORES = 8


def build_nc(n=N_FULL, mm=MM_FULL):
    """Build the per-core Bass program (SPMD: same program, per-core data)."""
    CI = IN_DIM // P          # 8 input-dim chunks
    CP = CI // 2              # c-chunk pairs for DR projections (4)
    JC = n // P               # key chunks (32)
    IG = n // FD              # query groups of 512 (8)
    MJ = mm // P              # masked key chunks (16)
    MG = mm // FD             # masked query groups (4)
    assert MJ % 2 == 0 and JC % 2 == 0
    AF = mybir.ActivationFunctionType

    nc = bacc.Bacc("TRN2", target_bir_lowering=False, debug=False)
    wq_d = nc.dram_tensor("wq", [P, CI * NH * DH], FP8, kind="ExternalInput")
    wk_d = nc.dram_tensor("wk", [P, CI * NH * DH], FP8, kind="ExternalInput")
    wv_d = nc.dram_tensor("wv", [P, CI * NH * DH], BF16, kind="ExternalInput")
    wo_d = nc.dram_tensor("wo", [P, NH * OUT_DIM], BF16, kind="ExternalInput")
    x8_d = nc.dram_tensor("x8", [IN_DIM, n], FP8, kind="ExternalInput")
    mk_d = nc.dram_tensor("maskt", [mm, mm], FP8, kind="ExternalInput")
    out_d = nc.dram_tensor("part", [n, OUT_DIM], BF16, kind="ExternalOutput")

    NQ = n // 4               # x8 DMA quarter width
    x8_v = x8_d.rearrange("(c p) n -> c p n", p=P)
    mk_v = mk_d.rearrange("(j p) i -> p j i", p=P)
    out_v = out_d.rearrange("(t p) o -> t p o", p=P)

    with tile.TileContext(nc) as tc, ExitStack() as ctx:
        const = ctx.enter_context(tc.tile_pool(name="const", bufs=1))

        # Resident inputs. Transfers are sliced small so they parallelize
        # across the 16 DMA engines (a single dma_start runs on ONE engine
        # at ~22GB/s), and the issue stream is split across the two HWDGE
        # queues (Sync + Scalar) because each dma_start costs ~0.6us of
        # issue time on its queue.  Sync: W slices + x8 first half.
        # Scalar (idle until the first exp): x8 second half.
        # x8 lives as four separate n-quarter tiles: the Tile dependency
        # tracker is per-tile, so a consumer of quarter q must not be made
        # to wait on later quarters' transfers.
        wq8 = const.tile([P, CI, NH * DH], FP8, tag="wq")
        wk8 = const.tile([P, CI, NH * DH], FP8, tag="wk")
        wq_v = wq_d.rearrange("p (a b) -> p a b", a=CI)
        wk_v = wk_d.rearrange("p (a b) -> p a b", a=CI)
        x8q = [const.tile([P, CI, NQ], FP8, tag=f"x8q{q}", name=f"x8q{q}")
               for q in range(4)]
        # first-needed pieces split across BOTH issue queues (each
        # dma_start costs ~0.6us of issue time on its queue)
        wv = const.tile([P, CI, NH * DH], BF16, tag="wv")
        wo = const.tile([P, NH, OUT_DIM], BF16, tag="wo")
        wv_v = wv_d.rearrange("p (a b) -> p a b", a=CI)
        for c in range(CI):
            (nc.sync if c < 4 else nc.scalar).dma_start(
                wq8[:, c, :], wq_v[:, c, :])
        for c in range(CI):
            (nc.sync if c < 4 else nc.scalar).dma_start(
                x8q[0][:, c, :], x8_v[c][:, 0:NQ])
        for c in range(0, CI, 4):
            nc.scalar.dma_start(wk8[:, c:c + 4, :], wk_v[:, c:c + 4, :])
        for c in range(CI):
            (nc.sync if c < 4 else nc.scalar).dma_start(
                x8q[1][:, c, :], x8_v[c][:, NQ:2 * NQ])
        for c in range(0, CI, 2):
            nc.sync.dma_start(wv[:, c:c + 2, :], wv_v[:, c:c + 2, :])
        for c in range(CI):
            (nc.sync if c < 4 else nc.scalar).dma_start(
                x8q[2][:, c, :], x8_v[c][:, 2 * NQ:3 * NQ])
        for c in range(CI):
            (nc.sync if c < 4 else nc.scalar).dma_start(
                x8q[3][:, c, :], x8_v[c][:, 3 * NQ:4 * NQ])
        nc.sync.dma_start(wo[:], wo_d.rearrange("p (a b) -> p a b", a=NH))

        def x8_slice(cp, lo, width):
            # [P, 2, width] view of columns lo..lo+width at c-pair cp
            q = lo // NQ
            assert (lo + width - 1) // NQ == q
            return x8q[q][:, 2 * cp:2 * cp + 2, lo - q * NQ:lo - q * NQ + width]
        ones8 = const.tile([P, 2, P], FP8, tag="ones")
        nc.vector.memset(ones8[:], 1.0)

        # Resident intermediates
        qt = [const.tile([P, n], BF16, tag=f"qt{h}", name=f"qt{h}") for h in range(NH)]
        kt = [const.tile([P, n], BF16, tag=f"kt{h}", name=f"kt{h}") for h in range(NH)]
        vb8 = const.tile([P, JC, NH * DH], FP8, tag="vb")      # [j, jc, (h d)]
        ot = [const.tile([P, n], BF16, tag=f"ot{h}", name=f"ot{h}") for h in range(NH)]

        pst = ctx.enter_context(tc.tile_pool(name="pst", bufs=2, space="PSUM"))
        px = ctx.enter_context(tc.tile_pool(name="px", bufs=2, space="PSUM"))
        po = ctx.enter_context(tc.tile_pool(name="po", bufs=1, space="PSUM"))
        pd = ctx.enter_context(tc.tile_pool(name="pd", bufs=1, space="PSUM"))
        att = ctx.enter_context(tc.tile_pool(name="att", bufs=5))
        mkp = ctx.enter_context(tc.tile_pool(name="mkp", bufs=4))
        obp = ctx.enter_context(tc.tile_pool(name="obp", bufs=3))

        # ---- emission units (each: a few PE streams + a DVE eviction) ----
        def emit_qk_g(h, w8, dst, g):
            # one i-group of a Q^T/K^T projection: DR over c-chunk pairs
            ps = px.tile([P, FD], F32, tag="u", name="psu")
            for cp in range(CP):
                nc.tensor.matmul(
                    ps[:], w8[:, 2 * cp:2 * cp + 2, h * DH:(h + 1) * DH],
                    x8_slice(cp, g * FD, FD),
                    start=(cp == 0), stop=(cp == CP - 1), perf_mode=DR,
                )
            nc.vector.tensor_copy(dst[:, g * FD:(g + 1) * FD], ps[:])

        def emit_v_chunk(t):
            # one 128-row chunk of V for both heads, evicted to fp8.
            # lhsT is the fp8 x (the PE takes mixed fp8 weights x bf16
            # ifmap); wv stays bf16 so V only carries x's quantization.
            ps = px.tile([P, FD], F32, tag="u", name="psu")
            pv = ps[:, :NH * DH]
            q, col = t * P // NQ, t * P % NQ
            for c in range(CI):
                nc.tensor.matmul(
                    pv, x8q[q][:, c, col:col + P], wv[:, c, :],
                    start=(c == 0), stop=(c == CI - 1),
                )
            nc.vector.tensor_copy(vb8[:, t, :], pv)

        def emit_outproj_half(t, nf):
            ps = px.tile([P, FD], F32, tag="u", name="psu")
            for h in range(NH):
                nc.tensor.matmul(
                    ps[:], ot[h][:, t * P:(t + 1) * P],
                    wo[:, h, nf * FD:(nf + 1) * FD],
                    start=(h == 0), stop=(h == NH - 1),
                )
            ob = obp.tile([P, FD], BF16, tag="ob", name="ob")
            nc.vector.tensor_copy(ob[:], ps[:])
            nc.sync.dma_start(out_v[t][:, nf * FD:(nf + 1) * FD], ob[:])

        # splice queues, drained on a fixed schedule inside the pair loop
        qk1_units = deque()
        op_units = deque()

        # ---- head: Q/K projections for head 0, first V chunks ----
        for w8, dst in ((wq8, qt[0]), (wk8, kt[0])):
            for g in range(IG):
                emit_qk_g(0, w8, dst, g)
        V_UPFRONT = 4
        for t in range(V_UPFRONT):
            emit_v_chunk(t)
        v_todo = deque(range(V_UPFRONT, JC))
        for w8, dst in ((wq8, qt[1]), (wk8, kt[1])):
            for g in range(IG):
                qk1_units.append(lambda h=1, w8=w8, dst=dst, g=g:
                                 emit_qk_g(h, w8, dst, g))

        # ---- attention pair loop (phases interleaved via splice pops) ----
        NP2 = JC // 2
        h0_pair = [0]

        def pop_splices(h, g, jp):
            if h == 0 and g == 0:
                # V chunks just-in-time, one pair ahead of this PV stream
                for _ in range(2):
                    if v_todo:
                        emit_v_chunk(v_todo.popleft())
            elif h == 0:
                # head-1 Q/K projection spread evenly over these 112 pairs
                h0_pair[0] += 1
                if h0_pair[0] % 7 == 3 and qk1_units:
                    qk1_units.popleft()()
            else:
                # output projection halves, one group's lag behind finalize
                if jp % 2 == 1 and op_units:
                    op_units.popleft()()

        for h in range(NH):
            for g in range(IG):
                gs = g * FD
                oacc = po.tile([P, FD], F32, tag="po")   # [d, i] accum
                dacc = pd.tile([P, FD], F32, tag="pd")   # bcast denom accum

                def emit_s(jp):
                    st = pst.tile([P, 2, FD], F32, tag="st", name="st")
                    for u in range(2):
                        nc.tensor.matmul(
                            st[:, u, :],
                            kt[h][:, (2 * jp + u) * P:(2 * jp + u + 1) * P],
                            qt[h][:, gs:gs + FD],
                            start=True, stop=True,
                        )
                    return st

                st_next = emit_s(0)
                for jp in range(NP2):
                    st = st_next
                    if jp + 1 < NP2:
                        st_next = emit_s(jp + 1)
                    pop_splices(h, g, jp)
                    j0 = 2 * jp
                    masked = j0 + 1 < MJ and g < MG
                    pt2 = att.tile([P, 2, FD], FP8, tag="pt")
                    nc.scalar.activation(pt2[:], st[:], AF.Exp, scale=SCALE)
                    if masked:
                        mt2 = mkp.tile([P, 2, FD], FP8, tag="mt")
                        nc.gpsimd.dma_start(
                            mt2[:], mk_v[:, j0:j0 + 2, gs:gs + FD])
                        nc.vector.tensor_mul(
                            out=pt2[:], in0=pt2[:], in1=mt2[:])
                    last_pair = jp == NP2 - 1
                    nc.tensor.matmul(
                        oacc[:], vb8[:, j0:j0 + 2, h * DH:(h + 1) * DH],
                        pt2[:], start=(jp == 0), stop=last_pair,
                        perf_mode=DR,
                    )
                    nc.tensor.matmul(
                        dacc[:], ones8[:], pt2[:],
                        start=(jp == 0), stop=last_pair,
                        perf_mode=DR,
                    )
                # free the single-bank accumulators ASAP, then normalize
                osb = att.tile([P, FD], F32, tag="osb", name="osb", bufs=2)
                dsb = att.tile([P, FD], F32, tag="dsb", name="dsb", bufs=2)
                nc.vector.tensor_copy(osb[:], oacc[:])
                nc.vector.tensor_copy(dsb[:], dacc[:])
                rec = att.tile([P, FD], F32, tag="rec", name="rec", bufs=2)
                nc.vector.reciprocal_approx_fast(rec[:], dsb[:])
                nc.vector.tensor_mul(
                    out=ot[h][:, gs:gs + FD], in0=osb[:], in1=rec[:])
                if h == NH - 1:
                    for t in range(4 * g, 4 * g + 4):
                        for nf in range(OUT_DIM // FD):
                            op_units.append(lambda t=t, nf=nf:
                                            emit_outproj_half(t, nf))

        # ---- tail: drain remaining spliced work ----
        while v_todo:
            emit_v_chunk(v_todo.popleft())
        while qk1_units:
            qk1_units.popleft()()
        while op_units:
            op_units.popleft()()

    nc.compile()
    return nc


def make_core_inputs(x, W_qkv, W_out, mask, n=N_FULL, mm=MM_FULL):
    """Host-side shard prep: per-core input dicts (pre-transposed).

    W slices are delivered in the on-chip layout ([128, c*h*d] with the
    IN_DIM chunk index between partition and column) so the DMA is dense.
    x ships twice: fp8 (Q/K DoubleRow path) and bf16 (V path).
    """
    bf = ml_dtypes.bfloat16
    f8 = ml_dtypes.float8_e4m3
    B = x.shape[0]
    CI = IN_DIM // P
    xt_f8 = [np.ascontiguousarray(x[b].T).astype(f8) for b in range(B)]
    maskt = np.ascontiguousarray(mask[0, 0, :mm, :mm].T).astype(f8)

    def wlayout(w, dt):  # [IN_DIM, NH*DH] -> [P, CI*NH*DH]
        return np.ascontiguousarray(
            w.reshape(CI, P, NH * DH).transpose(1, 0, 2).reshape(P, -1)
        ).astype(dt)

    cores_per_b = N_CORES // B
    in_maps = []
    for core in range(N_CORES):
        b = core // cores_per_b
        h0 = NH * (core % cores_per_b)
        qs, ks, vs = (W_qkv[:, o + h0 * DH: o + (h0 + NH) * DH]
                      for o in (0, OUT_DIM, 2 * OUT_DIM))
        wo_slice = W_out[h0 * DH:(h0 + NH) * DH, :]  # [NH*DH, OUT_DIM]
        wo_l = np.ascontiguousarray(
            wo_slice.reshape(NH, P, OUT_DIM).transpose(1, 0, 2).reshape(P, -1)
        ).astype(bf)
        in_maps.append({
            "x8": xt_f8[b],
            "wq": wlayout(qs, f8),
            "wk": wlayout(ks, f8),
            "wv": wlayout(vs, bf),
            "wo": wo_l,
            "maskt": maskt,
        })
    return in_maps


_NC_CACHE = {}


def _get_nc(n=N_FULL, mm=MM_FULL):
    key = (n, mm)
    if key not in _NC_CACHE:
        _NC_CACHE[key] = build_nc(n, mm)
    return _NC_CACHE[key]


def run(x, W_qkv, W_out, b_out, mask, trace=False, **trace_kwargs):
    nc = _get_nc()
    in_maps = make_core_inputs(x, W_qkv, W_out, mask)
    res = run_bass_kernel_spmd(
        nc, in_maps, list(range(N_CORES)), trace=trace, **trace_kwargs
    )
    B = x.shape[0]
    cores_per_b = N_CORES // B
    out = np.zeros((B, N_FULL, OUT_DIM), np.float32)
    for core in range(N_CORES):
        out[core // cores_per_b] += np.asarray(
            res.results[core]["part"], dtype=np.float32)
    out += np.asarray(b_out, np.float32)
    return out, res


def kernel(x, W_qkv, W_out, b_out, mask, max_mask=MM_FULL, **_ignored):
    x = np.asarray(x, np.float32)
    W_qkv = np.asarray(W_qkv, np.float32)
    W_out = np.asarray(W_out, np.float32)
    b_out = np.asarray(b_out, np.float32)
    mask = np.asarray(mask)
    out, _ = run(x, W_qkv, W_out, b_out, mask)
    return out
